# revision 20
# baseline (speedup 1.0000x reference)
"""Multi-head attention (RoPE) Trainium2 kernel, 8-way sharded.

Sharding: core c handles batch b = c//4 and 4 heads h0 = 4*(c%4).

Per-core device program (v2 — HAM-warm dense PE stream):

  inputs (per core, fp16 except noted):
    xT   [1024, 2048]  = x[b].T
    wqkT [1024, 512]   = concat(w_q_rows, w_k_rows).T
    wvT  [1024, 256]   = w_v_rows.T
    woT  [256, 1024]   = w_out[:, head_cols].T
    cosT [128, 2048]   rope cos table, 2 heads stacked
    sinT [128, 2048]   rope sin table, rotate-half sign baked in

  program:
    qk_proj: qkT = wqk.T @ xT + rope     [512, 2048] feat-major fp16
             (rope: ACT copies PSUM->fp16, GpSimd does the 4 rotate-half
              mults, DVE does cos-mult + add — all off the PE)
    v_proj:  V' = x @ w_v.T (+ ones col) [2048, 4*65] token-major fp16
    attention per (q-half, head), software-pipelined S(kt+1) ahead of
    PV(kt) so the PE never idles (keeps HAM at K=8/8):
      S^T[kt]  = k'T.T @ q'T      [128, 1024] PSUM (mm tag, 2 bufs)
      es[kt]   = exp(0.125 S^T)   [128, 1024] fp16 (ACT, 3 bufs)
      pv      += [V|1].T @ es     [65, 1024] PSUM (pv tag, 2 bufs)
      epilogue: recip(pv[64]) bcast mult -> ao fp16
    out_proj(q-half): y = ao.T @ wo      [2048, 1024] fp16 partial
  host: y[b] = fp32 sum of the 4 per-core partials.
"""

import numpy as np

B = 2
N = 2048
C = 1024
H_TOT = 16
HD = 64
HC = 4  # heads per core
N_CORES = 8
ROPE_BASE = 10000.0

_PROGRAM = None


def _rope_tables():
    inv_freq = 1.0 / (ROPE_BASE ** (np.arange(0, HD, 2, dtype=np.float32) / HD))
    t = np.arange(N, dtype=np.float32)
    freqs = np.einsum("i,j->ij", t, inv_freq).astype(np.float32)  # [N, 32]
    emb = np.concatenate([freqs, freqs], axis=-1)  # [N, 64]
    cos = np.cos(emb).astype(np.float32)
    sin = np.sin(emb).astype(np.float32)
    cosT = np.ascontiguousarray(np.tile(cos.T, (2, 1)))  # [128, 2048]
    sinT = sin.T.copy()  # [64, 2048]
    sinT_signed = np.concatenate([-sinT[:32], sinT[32:]], axis=0)
    # row-swapped so u[o:o+32] = t16[i:i+32] * sinT2[i:i+32] (equal input
    # base partitions — a walrus requirement when both inputs are in SBUF)
    sinT_shuf = np.concatenate([sinT_signed[32:], sinT_signed[:32]], axis=0)
    sinT2 = np.ascontiguousarray(np.tile(sinT_shuf, (2, 1)))  # [128, 2048]
    return cosT.astype(np.float16), sinT2.astype(np.float16)


def _build_program(debug=False):
    import concourse.mybir as mybir
    import concourse.tile as tile
    from concourse import bacc

    f32 = mybir.dt.float32
    f16 = mybir.dt.float16
    i16 = mybir.dt.int16
    MUL = mybir.AluOpType.mult
    ADD = mybir.AluOpType.add
    EXP = mybir.ActivationFunctionType.Exp
    # Schraudolph fp16-exp constants: bits(exp(S/8)) ~= A*S + B
    SCHRAUD_A = 1024.0 * 1.4426950408889634 * float(HD**-0.5)
    SCHRAUD_B = 1024.0 * 15.0 - 68.0

    nc = bacc.Bacc("TRN2", target_bir_lowering=False, debug=False, num_devices=N_CORES)

    xT_d = nc.dram_tensor("xT", [C, N], f16, kind="ExternalInput").ap()
    wqk_d = nc.dram_tensor("wqkT", [C, 2 * HC * HD], f16, kind="ExternalInput").ap()
    wv_d = nc.dram_tensor("wvT", [C, HC * HD], f16, kind="ExternalInput").ap()
    wo_d = nc.dram_tensor("woT", [HC * HD, C], f16, kind="ExternalInput").ap()
    cos_d = nc.dram_tensor("cosT", [128, N], f16, kind="ExternalInput").ap()
    sin_d = nc.dram_tensor("sinT", [128, N], f16, kind="ExternalInput").ap()
    y_d = nc.dram_tensor("y", [N, C], f16, kind="ExternalOutput").ap()
    if debug:
        qk_dbg = nc.dram_tensor("qk_dbg", [4, 128, N], f16, kind="ExternalOutput").ap()
        vv_dbg = nc.dram_tensor(
            "vv_dbg", [128, 16 * HC * (HD + 1)], f16, kind="ExternalOutput"
        ).ap()
        ao_dbg = nc.dram_tensor("ao_dbg", [2, 128, N], f16, kind="ExternalOutput").ap()

    with tile.TileContext(nc) as tc:
        with (
            tc.tile_pool(name="persist", bufs=1) as persist,
            tc.tile_pool(name="work", bufs=2) as work,
            tc.tile_pool(name="psum", bufs=2, space="PSUM") as psp,
        ):
            xT = persist.tile([128, 8, N], f16, tag="xT", name="xT")
            wqk = persist.tile([128, 8, 2 * HC * HD], f16, tag="wqk", name="wqk")
            wv = persist.tile([128, 8, HC * HD], f16, tag="wv", name="wv")
            wo = persist.tile([128, 2, C], f16, tag="wo", name="wo")
            cosT = persist.tile([128, N], f16, tag="cosT", name="cosT")
            sinT = persist.tile([128, N], f16, tag="sinT", name="sinT")
            qk = [
                persist.tile([128, N], f16, tag=f"qk{i}", name=f"qk{i}")
                for i in range(4)
            ]
            vv = persist.tile([128, 16, HC, HD + 1], f16, tag="vv", name="vv")
            ao = [
                persist.tile([128, N], f16, tag=f"ao{i}", name=f"ao{i}")
                for i in range(2)
            ]

            # -- input DMA: split across the two HWDGE queues (sync, scalar),
            # xT per contraction-chunk so the first qk matmuls start early
            for ct in range(8):
                eng = nc.sync if ct % 2 == 0 else nc.scalar
                eng.dma_start(wqk[:, ct, :], wqk_d[ct * 128 : (ct + 1) * 128, :])
            for ct in range(8):
                eng = nc.sync if ct % 2 == 0 else nc.scalar
                eng.dma_start(xT[:, ct, :], xT_d[ct * 128 : (ct + 1) * 128, :])
            nc.scalar.dma_start(cosT[:], cos_d[:, :])
            nc.scalar.dma_start(sinT[:], sin_d[:, :])
            nc.sync.dma_start(wv[:], wv_d.rearrange("(i p) m -> p i m", p=128))
            nc.sync.dma_start(wo[:], wo_d.rearrange("(i p) m -> p i m", p=128))

            # preload the exp ACT table during the projection phase
            warm = work.tile([1, 16], f16, tag="warm", name="warm", bufs=1)
            nc.scalar.activation(warm[:], cosT[0:1, 0:16], EXP)

            def qk_unit(pt, half):
                """project+rope one [128-feat, 1024-token] chunk of q/k."""
                hs = slice(half * 1024, (half + 1) * 1024)
                bp = psp.tile([128, 1024], f32, tag="mm", name=f"bp{pt}_{half}")
                for ct in range(8):
                    for s2 in range(2):
                        so = slice(s2 * 512, (s2 + 1) * 512)
                        si = slice(half * 1024 + s2 * 512, half * 1024 + (s2 + 1) * 512)
                        nc.tensor.matmul(
                            bp[:, so],
                            wqk[:, ct, pt * 128 : (pt + 1) * 128],
                            xT[:, ct, si],
                            start=(ct == 0),
                            stop=(ct == 7),
                        )
                t16 = work.tile([128, 1024], f16, tag="t16", name="t16", bufs=3)
                nc.scalar.copy(t16[:], bp[:])
                # u = t16 * sin (sign+shuffle baked into the table), then the
                # rotate-half partition swap rides the idle DMA queues
                u = work.tile([128, 1024], f16, tag="u", name="u")
                nc.vector.tensor_tensor(u[:], t16[:], sinT[:, hs], MUL)
                u2 = work.tile([128, 1024], f16, tag="u2", name="u2")
                for qi, (o, i) in enumerate(((0, 32), (32, 0), (64, 96), (96, 64))):
                    eng = nc.sync if qi % 2 == 0 else nc.scalar
                    eng.dma_start(u2[o : o + 32, :], u[i : i + 32, :])
                t2 = work.tile([128, 1024], f16, tag="t2", name="t2")
                nc.gpsimd.tensor_tensor(t2[:], t16[:], cosT[:, hs], MUL)
                nc.vector.tensor_tensor(qk[pt][:, hs], t2[:], u2[:], ADD)

            def v_unit(tt):
                """V' tile for one 128-token block (token-major) + ones col."""
                vp = psp.tile([128, HC * HD], f32, tag="mm", name=f"vp{tt}")
                for ct in range(8):
                    nc.tensor.matmul(
                        vp[:, :],
                        xT[:, ct, tt * 128 : (tt + 1) * 128],
                        wv[:, ct, :],
                        start=(ct == 0),
                        stop=(ct == 7),
                    )
                nc.scalar.copy(
                    vv[:, tt, :, 0:HD],
                    vp[:].rearrange("p (h d) -> p h d", h=HC),
                )

            def attention(qh, h):
                """one head, one 1024-query half; dense pipelined PE stream."""
                qpt = h // 2
                roff = 64 * (h % 2)
                qs = slice(qh * 1024, (qh + 1) * 1024)
                pv = psp.tile([65, 1024], f32, tag="pv", name=f"pv{qh}_{h}")

                def s_mm(kt):
                    sp = psp.tile([128, 1024], f32, tag="mm", name=f"sp{qh}{h}{kt}")
                    for s2 in range(2):
                        nc.tensor.matmul(
                            sp[:, s2 * 512 : (s2 + 1) * 512],
                            qk[2 + qpt][roff : roff + 64, kt * 128 : (kt + 1) * 128],
                            qk[qpt][
                                roff : roff + 64,
                                qh * 1024 + s2 * 512 : qh * 1024 + (s2 + 1) * 512,
                            ],
                            start=True,
                            stop=True,
                        )
                    return sp

                def e_act(kt, sp):
                    if kt % 4 == 3:
                        # Schraudolph exp on DVE: fp16 bit-pattern = A*x + B,
                        # rounded to int16, bitcast to fp16 (offloads ~25% of
                        # the exp stream from the saturated ACT engine)
                        ts = work.tile([128, 1024], f32, tag="ts8", name="ts8")
                        nc.vector.tensor_scalar(
                            ts[:], sp[:], SCHRAUD_A, SCHRAUD_B, MUL, ADD
                        )
                        esi = work.tile([128, 1024], i16, tag="es8", name="es8")
                        nc.vector.tensor_copy(esi[:], ts[:])
                        return esi[:].bitcast(f16)
                    es = work.tile([128, 1024], f16, tag="es", bufs=4, name="es")
                    nc.scalar.activation(es[:], sp[:], EXP, scale=float(HD**-0.5))
                    return es[:]

                def pv_mm(kt, es):
                    for s2 in range(2):
                        nc.tensor.matmul(
                            pv[:, s2 * 512 : (s2 + 1) * 512],
                            vv[:, kt, h, :],
                            es[:, s2 * 512 : (s2 + 1) * 512],
                            start=(kt == 0),
                            stop=(kt == 15),
                        )

                sp_prev = s_mm(0)
                for kt in range(1, 16):
                    sp_cur = s_mm(kt)
                    pv_mm(kt - 1, e_act(kt - 1, sp_prev))
                    sp_prev = sp_cur
                pv_mm(15, e_act(15, sp_prev))

                rr = work.tile([1, 1024], f32, tag="rr", name="rr")
                ra = work.tile([1, 1024], f32, tag="ra", name="ra")
                nb = work.tile([64, 1024], f32, tag="nb", name="nb")
                nc.vector.tensor_copy(rr[:], pv[64:65, :])
                nc.vector.reciprocal_approx_fast(ra[:], rr[:])
                nc.gpsimd.partition_broadcast(nb[:], ra[:])
                nc.vector.tensor_tensor(
                    ao[qpt][roff : roff + 64, qs], pv[0:64, :], nb[:], MUL
                )

            def out_unit(tt, tail=False):
                yps = psp.tile([128, C], f32, tag="pv", name=f"yps{tt}")
                for oc in range(2):
                    osl = slice(oc * 512, (oc + 1) * 512)
                    for ft in range(2):
                        nc.tensor.matmul(
                            yps[:, osl],
                            ao[ft][:, tt * 128 : (tt + 1) * 128],
                            wo[:, ft, osl],
                            start=(ft == 0),
                            stop=(ft == 1),
                        )
                ysb = work.tile([128, C], f16, tag="ysb", bufs=3, name="ysb")
                if tt % 2 == 0:
                    nc.scalar.copy(ysb[:], yps[:])
                else:
                    nc.vector.tensor_copy(ysb[:], yps[:])
                eng = nc.sync if tt % 2 == 0 else nc.scalar
                eng.dma_start(y_d[tt * 128 : (tt + 1) * 128, :], ysb[:])

            for tt in range(16):
                nc.vector.memset(vv[:, tt, :, HD], 1.0)
            for pt in (0, 2, 1, 3):
                for half in (0, 1):
                    qk_unit(pt, half)
            for tt in range(16):
                v_unit(tt)
            for qh in range(2):
                for h in range(4):
                    attention(qh, h)
                for tt in range(qh * 8, qh * 8 + 8):
                    out_unit(tt, tail=(qh == 1))
            if debug:
                for pt in range(4):
                    nc.sync.dma_start(qk_dbg[pt], qk[pt][:])
                nc.sync.dma_start(vv_dbg[:, :], vv[:].rearrange("p a b c -> p (a b c)"))
                for i in range(2):
                    nc.sync.dma_start(ao_dbg[i], ao[i][:])

    nc.compile()
    return nc


def _get_program():
    global _PROGRAM
    if _PROGRAM is None:
        _PROGRAM = _build_program()
    return _PROGRAM


def _make_in_maps(x, w_qkv, w_out):
    x = np.asarray(x, dtype=np.float32)
    w_qkv = np.asarray(w_qkv, dtype=np.float32)
    w_out = np.asarray(w_out, dtype=np.float32)
    cosT, sinT = _rope_tables()
    in_maps = []
    for c in range(N_CORES):
        b = c // 4
        h0 = HC * (c % 4)
        rows = np.arange(h0 * HD, (h0 + HC) * HD)
        wq = w_qkv[rows]  # [256, 1024]
        wk = w_qkv[C + rows]
        wv = w_qkv[2 * C + rows]
        in_maps.append(
            {
                "xT": np.ascontiguousarray(x[b].T).astype(np.float16),
                "wqkT": np.ascontiguousarray(np.concatenate([wq, wk], 0).T).astype(
                    np.float16
                ),
                "wvT": np.ascontiguousarray(wv.T).astype(np.float16),
                "woT": np.ascontiguousarray(w_out[:, rows].T).astype(np.float16),
                "cosT": cosT,
                "sinT": sinT,
            }
        )
    return in_maps


def run(inputs, trace=False, trace_cores=None):
    from concourse.bass_utils import run_bass_kernel_spmd

    nc = _get_program()
    in_maps = _make_in_maps(inputs["x"], inputs["w_qkv"], inputs["w_out"])
    res = run_bass_kernel_spmd(
        nc,
        in_maps,
        core_ids=list(range(N_CORES)),
        trace=trace,
        trace_cores=trace_cores,
    )
    y = np.zeros((B, N, C), dtype=np.float32)
    for c in range(N_CORES):
        y[c // 4] += res.results[c]["y"].astype(np.float32)
    return y, res


def kernel(**inputs) -> np.ndarray:
    y, _ = run(inputs, trace=False)
    return y


# revision 21
# speedup vs baseline: 1.0379x; 1.0379x over previous
"""Multi-head attention (RoPE) Trainium2 kernel, 8-way sharded.

Sharding: core c handles batch b = c//4 and 4 heads h0 = 4*(c%4).

Per-core device program (v2 — HAM-warm dense PE stream):

  inputs (per core, fp16 except noted):
    xT   [1024, 2048]  = x[b].T
    wqkT [1024, 512]   = concat(w_q_rows, w_k_rows).T
    wvT  [1024, 256]   = w_v_rows.T
    woT  [256, 1024]   = w_out[:, head_cols].T
    cosT [128, 2048]   rope cos table, 2 heads stacked
    sinT [128, 2048]   rope sin table, rotate-half sign baked in

  program:
    qk_proj: qkT = wqk.T @ xT + rope     [512, 2048] feat-major fp16
             (rope: ACT copies PSUM->fp16, GpSimd does the 4 rotate-half
              mults, DVE does cos-mult + add — all off the PE)
    v_proj:  V' = x @ w_v.T (+ ones col) [2048, 4*65] token-major fp16
    attention per (q-half, head), software-pipelined S(kt+1) ahead of
    PV(kt) so the PE never idles (keeps HAM at K=8/8):
      S^T[kt]  = k'T.T @ q'T      [128, 1024] PSUM (mm tag, 2 bufs)
      es[kt]   = exp(0.125 S^T)   [128, 1024] fp16 (ACT, 3 bufs)
      pv      += [V|1].T @ es     [65, 1024] PSUM (pv tag, 2 bufs)
      epilogue: recip(pv[64]) bcast mult -> ao fp16
    out_proj(q-half): y = ao.T @ wo      [2048, 1024] fp16 partial
  host: y[b] = fp32 sum of the 4 per-core partials.
"""

import numpy as np

B = 2
N = 2048
C = 1024
H_TOT = 16
HD = 64
HC = 4  # heads per core
N_CORES = 8
ROPE_BASE = 10000.0

_PROGRAM = None


def _rope_tables():
    inv_freq = 1.0 / (ROPE_BASE ** (np.arange(0, HD, 2, dtype=np.float32) / HD))
    t = np.arange(N, dtype=np.float32)
    freqs = np.einsum("i,j->ij", t, inv_freq).astype(np.float32)  # [N, 32]
    emb = np.concatenate([freqs, freqs], axis=-1)  # [N, 64]
    cos = np.cos(emb).astype(np.float32)
    sin = np.sin(emb).astype(np.float32)
    cosT = np.ascontiguousarray(np.tile(cos.T, (2, 1)))  # [128, 2048]
    sinT = sin.T.copy()  # [64, 2048]
    sinT_signed = np.concatenate([-sinT[:32], sinT[32:]], axis=0)
    # row-swapped so u[o:o+32] = t16[i:i+32] * sinT2[i:i+32] (equal input
    # base partitions — a walrus requirement when both inputs are in SBUF)
    sinT_shuf = np.concatenate([sinT_signed[32:], sinT_signed[:32]], axis=0)
    sinT2 = np.ascontiguousarray(np.tile(sinT_shuf, (2, 1)))  # [128, 2048]
    return cosT.astype(np.float16), sinT2.astype(np.float16)


def _build_program(debug=False):
    import concourse.mybir as mybir
    import concourse.tile as tile
    from concourse import bacc

    f32 = mybir.dt.float32
    f16 = mybir.dt.float16
    i16 = mybir.dt.int16
    MUL = mybir.AluOpType.mult
    ADD = mybir.AluOpType.add
    EXP = mybir.ActivationFunctionType.Exp
    # Schraudolph fp16-exp constants: bits(exp(S/8)) ~= A*S + B
    SCHRAUD_A = 1024.0 * 1.4426950408889634 * float(HD**-0.5)
    SCHRAUD_B = 1024.0 * 15.0 - 68.0

    nc = bacc.Bacc("TRN2", target_bir_lowering=False, debug=False, num_devices=N_CORES)

    xT_d = nc.dram_tensor("xT", [C, N], f16, kind="ExternalInput").ap()
    wqk_d = nc.dram_tensor("wqkT", [C, 2 * HC * HD], f16, kind="ExternalInput").ap()
    wv_d = nc.dram_tensor("wvT", [C, HC * HD], f16, kind="ExternalInput").ap()
    wo_d = nc.dram_tensor("woT", [HC * HD, C], f16, kind="ExternalInput").ap()
    cos_d = nc.dram_tensor("cosT", [128, N], f16, kind="ExternalInput").ap()
    sin_d = nc.dram_tensor("sinT", [128, N], f16, kind="ExternalInput").ap()
    y_d = nc.dram_tensor("y", [N, C], f16, kind="ExternalOutput").ap()
    if debug:
        qk_dbg = nc.dram_tensor("qk_dbg", [4, 128, N], f16, kind="ExternalOutput").ap()
        vv_dbg = nc.dram_tensor(
            "vv_dbg", [128, 16 * HC * (HD + 1)], f16, kind="ExternalOutput"
        ).ap()
        ao_dbg = nc.dram_tensor("ao_dbg", [2, 128, N], f16, kind="ExternalOutput").ap()

    with tile.TileContext(nc) as tc:
        with (
            tc.tile_pool(name="persist", bufs=1) as persist,
            tc.tile_pool(name="work", bufs=2) as work,
            tc.tile_pool(name="psum", bufs=2, space="PSUM") as psp,
        ):
            xT = persist.tile([128, 8, N], f16, tag="xT", name="xT")
            wqk = persist.tile([128, 8, 2 * HC * HD], f16, tag="wqk", name="wqk")
            wv = persist.tile([128, 8, HC * HD], f16, tag="wv", name="wv")
            wo = persist.tile([128, 2, C], f16, tag="wo", name="wo")
            cosT = persist.tile([128, N], f16, tag="cosT", name="cosT")
            sinT = persist.tile([128, N], f16, tag="sinT", name="sinT")
            qk = [
                persist.tile([128, N], f16, tag=f"qk{i}", name=f"qk{i}")
                for i in range(4)
            ]
            vv = persist.tile([128, 16, HC, HD + 1], f16, tag="vv", name="vv")
            ao = [
                persist.tile([128, N], f16, tag=f"ao{i}", name=f"ao{i}")
                for i in range(2)
            ]

            # -- input DMA: split across the two HWDGE queues (sync, scalar),
            # xT per contraction-chunk so the first qk matmuls start early
            for ct in range(8):
                eng = nc.sync if ct % 2 == 0 else nc.scalar
                eng.dma_start(wqk[:, ct, :], wqk_d[ct * 128 : (ct + 1) * 128, :])
            for ct in range(8):
                eng = nc.sync if ct % 2 == 0 else nc.scalar
                eng.dma_start(xT[:, ct, :], xT_d[ct * 128 : (ct + 1) * 128, :])
            nc.scalar.dma_start(cosT[:], cos_d[:, :])
            nc.scalar.dma_start(sinT[:], sin_d[:, :])
            nc.sync.dma_start(wv[:], wv_d.rearrange("(i p) m -> p i m", p=128))
            nc.sync.dma_start(wo[:], wo_d.rearrange("(i p) m -> p i m", p=128))

            # preload the exp ACT table during the projection phase
            warm = work.tile([1, 16], f16, tag="warm", name="warm", bufs=1)
            nc.scalar.activation(warm[:], cosT[0:1, 0:16], EXP)

            def qk_unit(pt, half):
                """project+rope one [128-feat, 1024-token] chunk of q/k."""
                hs = slice(half * 1024, (half + 1) * 1024)
                bp = psp.tile([128, 1024], f32, tag="mm", name=f"bp{pt}_{half}")
                for ct in range(8):
                    for s2 in range(2):
                        so = slice(s2 * 512, (s2 + 1) * 512)
                        si = slice(half * 1024 + s2 * 512, half * 1024 + (s2 + 1) * 512)
                        nc.tensor.matmul(
                            bp[:, so],
                            wqk[:, ct, pt * 128 : (pt + 1) * 128],
                            xT[:, ct, si],
                            start=(ct == 0),
                            stop=(ct == 7),
                        )
                t16 = work.tile([128, 1024], f16, tag="t16", name="t16", bufs=3)
                nc.scalar.copy(t16[:], bp[:])
                # u = t16 * sin (sign+shuffle baked into the table), then the
                # rotate-half partition swap rides the idle DMA queues
                u = work.tile([128, 1024], f16, tag="u", name="u")
                nc.vector.tensor_tensor(u[:], t16[:], sinT[:, hs], MUL)
                u2 = work.tile([128, 1024], f16, tag="u2", name="u2")
                for qi, (o, i) in enumerate(((0, 32), (32, 0), (64, 96), (96, 64))):
                    eng = nc.sync if qi % 2 == 0 else nc.scalar
                    eng.dma_start(u2[o : o + 32, :], u[i : i + 32, :])
                t2 = work.tile([128, 1024], f16, tag="t2", name="t2")
                nc.gpsimd.tensor_tensor(t2[:], t16[:], cosT[:, hs], MUL)
                nc.vector.tensor_tensor(qk[pt][:, hs], t2[:], u2[:], ADD)

            def v_unit(tt):
                """V' tile for one 128-token block (token-major) + ones col."""
                vp = psp.tile([128, HC * HD], f32, tag="mm", name=f"vp{tt}")
                for ct in range(8):
                    nc.tensor.matmul(
                        vp[:, :],
                        xT[:, ct, tt * 128 : (tt + 1) * 128],
                        wv[:, ct, :],
                        start=(ct == 0),
                        stop=(ct == 7),
                    )
                nc.scalar.copy(
                    vv[:, tt, :, 0:HD],
                    vp[:].rearrange("p (h d) -> p h d", h=HC),
                )

            def attention(qh, h):
                """one head, one 1024-query half; dense pipelined PE stream."""
                qpt = h // 2
                roff = 64 * (h % 2)
                qs = slice(qh * 1024, (qh + 1) * 1024)
                pv = psp.tile([65, 1024], f32, tag="pv", name=f"pv{qh}_{h}")

                def s_mm(kt):
                    sp = psp.tile([128, 1024], f32, tag="mm", name=f"sp{qh}{h}{kt}")
                    for s2 in range(2):
                        nc.tensor.matmul(
                            sp[:, s2 * 512 : (s2 + 1) * 512],
                            qk[2 + qpt][roff : roff + 64, kt * 128 : (kt + 1) * 128],
                            qk[qpt][
                                roff : roff + 64,
                                qh * 1024 + s2 * 512 : qh * 1024 + (s2 + 1) * 512,
                            ],
                            start=True,
                            stop=True,
                        )
                    return sp

                def e_act(kt, sp):
                    es = work.tile([128, 1024], f16, tag="es", bufs=4, name="es")
                    nc.scalar.activation(es[:], sp[:], EXP, scale=float(HD**-0.5))
                    return es[:]

                def pv_mm(kt, es):
                    for s2 in range(2):
                        nc.tensor.matmul(
                            pv[:, s2 * 512 : (s2 + 1) * 512],
                            vv[:, kt, h, :],
                            es[:, s2 * 512 : (s2 + 1) * 512],
                            start=(kt == 0),
                            stop=(kt == 15),
                        )

                sp_prev = s_mm(0)
                for kt in range(1, 16):
                    sp_cur = s_mm(kt)
                    pv_mm(kt - 1, e_act(kt - 1, sp_prev))
                    sp_prev = sp_cur
                pv_mm(15, e_act(15, sp_prev))

                rr = work.tile([1, 1024], f32, tag="rr", name="rr")
                ra = work.tile([1, 1024], f32, tag="ra", name="ra")
                nb = work.tile([64, 1024], f32, tag="nb", name="nb")
                nc.vector.tensor_copy(rr[:], pv[64:65, :])
                nc.vector.reciprocal_approx_fast(ra[:], rr[:])
                nc.gpsimd.partition_broadcast(nb[:], ra[:])
                nc.vector.tensor_tensor(
                    ao[qpt][roff : roff + 64, qs], pv[0:64, :], nb[:], MUL
                )

            def out_unit(tt, tail=False):
                yps = psp.tile([128, C], f32, tag="pv", name=f"yps{tt}")
                for oc in range(2):
                    osl = slice(oc * 512, (oc + 1) * 512)
                    for ft in range(2):
                        nc.tensor.matmul(
                            yps[:, osl],
                            ao[ft][:, tt * 128 : (tt + 1) * 128],
                            wo[:, ft, osl],
                            start=(ft == 0),
                            stop=(ft == 1),
                        )
                ysb = work.tile([128, C], f16, tag="ysb", bufs=3, name="ysb")
                if tt % 2 == 0:
                    nc.scalar.copy(ysb[:], yps[:])
                else:
                    nc.vector.tensor_copy(ysb[:], yps[:])
                eng = nc.sync if tt % 2 == 0 else nc.scalar
                eng.dma_start(y_d[tt * 128 : (tt + 1) * 128, :], ysb[:])

            for tt in range(16):
                nc.vector.memset(vv[:, tt, :, HD], 1.0)
            for pt in (0, 2, 1, 3):
                for half in (0, 1):
                    qk_unit(pt, half)
            for tt in range(16):
                v_unit(tt)
            for qh in range(2):
                for h in range(4):
                    attention(qh, h)
                for tt in range(qh * 8, qh * 8 + 8):
                    out_unit(tt, tail=(qh == 1))
            if debug:
                for pt in range(4):
                    nc.sync.dma_start(qk_dbg[pt], qk[pt][:])
                nc.sync.dma_start(vv_dbg[:, :], vv[:].rearrange("p a b c -> p (a b c)"))
                for i in range(2):
                    nc.sync.dma_start(ao_dbg[i], ao[i][:])

    nc.compile()
    return nc


def _get_program():
    global _PROGRAM
    if _PROGRAM is None:
        _PROGRAM = _build_program()
    return _PROGRAM


def _make_in_maps(x, w_qkv, w_out):
    x = np.asarray(x, dtype=np.float32)
    w_qkv = np.asarray(w_qkv, dtype=np.float32)
    w_out = np.asarray(w_out, dtype=np.float32)
    cosT, sinT = _rope_tables()
    in_maps = []
    for c in range(N_CORES):
        b = c // 4
        h0 = HC * (c % 4)
        rows = np.arange(h0 * HD, (h0 + HC) * HD)
        wq = w_qkv[rows]  # [256, 1024]
        wk = w_qkv[C + rows]
        wv = w_qkv[2 * C + rows]
        in_maps.append(
            {
                "xT": np.ascontiguousarray(x[b].T).astype(np.float16),
                "wqkT": np.ascontiguousarray(np.concatenate([wq, wk], 0).T).astype(
                    np.float16
                ),
                "wvT": np.ascontiguousarray(wv.T).astype(np.float16),
                "woT": np.ascontiguousarray(w_out[:, rows].T).astype(np.float16),
                "cosT": cosT,
                "sinT": sinT,
            }
        )
    return in_maps


def run(inputs, trace=False, trace_cores=None):
    from concourse.bass_utils import run_bass_kernel_spmd

    nc = _get_program()
    in_maps = _make_in_maps(inputs["x"], inputs["w_qkv"], inputs["w_out"])
    res = run_bass_kernel_spmd(
        nc,
        in_maps,
        core_ids=list(range(N_CORES)),
        trace=trace,
        trace_cores=trace_cores,
    )
    y = np.zeros((B, N, C), dtype=np.float32)
    for c in range(N_CORES):
        y[c // 4] += res.results[c]["y"].astype(np.float32)
    return y, res


def kernel(**inputs) -> np.ndarray:
    y, _ = run(inputs, trace=False)
    return y
